# revision 1
# baseline (speedup 1.0000x reference)
"""GNN message-passing (SpMM + dense transform) Trainium2 kernel.

out[i] = (sum_{e: row[e]==i} vals[e] * x[col[e]]) @ W + b

Strategy (8 NeuronCores, SPMD single program):
- Host packs nodes into 1600 blocks (<=64 nodes, <=640 edges each) via LPT
  bin-packing; 200 blocks per core; each block = 5 chunks of 128 edge slots.
- Per chunk: indirect-DMA gather of 128 x-rows (one per partition), a DVE
  tensor_scalar builds a vals-weighted one-hot [128, 64] from a constant
  iota, and a fp32 matmul accumulates accT[64 feats, 64 rows] in PSUM.
- Per block: ACT evacuates accT, one matmul with W (outT = W.T @ accT),
  ACT adds bias, DMA out. Host unpermutes rows at the end.
"""
import sys
import heapq

for _p in ("/opt/trn_rl_repo", "/root/.axon_site/_ro/trn_rl_repo"):
    if _p not in sys.path:
        sys.path.append(_p)

import numpy as np

N_NODES = 100000
N_EDGES = 1000000
F = 64
P = 128
W_R = 64          # rows per block
CPB = 5           # chunks per block
EPB = CPB * P     # edge slots per block = 640
NBLK = 1600       # total blocks
NCORE = 8
BPC = NBLK // NCORE   # blocks per core = 200
NCH = BPC * CPB       # chunks per core = 1000

_cache = {}
LAST = {}  # debug/profiling handle: {"nc": ..., "in_maps": [...]}


def _build_program():
    import concourse.bass as bass
    import concourse.bacc as bacc
    import concourse.mybir as mybir
    import concourse.tile as tile

    nc = bacc.Bacc(trn_type="TRN2", dynamic_dma_scratch_size=65536)
    f32 = mybir.dt.float32
    d_x = nc.declare_dram_parameter("x", [N_NODES, F], f32, isOutput=False)
    d_gidx = nc.declare_dram_parameter("gidx", [P, NCH], mybir.dt.int32, isOutput=False)
    d_rl = nc.declare_dram_parameter("rl", [P, NCH], f32, isOutput=False)
    d_vals = nc.declare_dram_parameter("vals", [P, NCH], f32, isOutput=False)
    d_iota = nc.declare_dram_parameter("iota", [P, W_R + 1], f32, isOutput=False)
    d_W = nc.declare_dram_parameter("W", [F, F], f32, isOutput=False)
    d_b = nc.declare_dram_parameter("b", [F, 1], f32, isOutput=False)
    d_out = nc.declare_dram_parameter("out", [BPC, F, W_R], f32, isOutput=True)

    with tile.TileContext(nc) as tc:
        with (
            tc.tile_pool(name="const", bufs=1) as constp,
            tc.tile_pool(name="g", bufs=16) as gp,
            tc.tile_pool(name="oh", bufs=8) as ohp,
            tc.tile_pool(name="ev", bufs=4) as evp,
            tc.tile_pool(name="accp", bufs=2, space="PSUM") as accp,
            tc.tile_pool(name="outp", bufs=2, space="PSUM") as outpp,
        ):
            t_gidx = constp.tile([P, NCH], mybir.dt.int32)
            t_rl = constp.tile([P, NCH], f32)
            t_vals = constp.tile([P, NCH], f32)
            t_iota = constp.tile([P, W_R + 1], f32)
            t_W = constp.tile([F, F], f32)
            t_b = constp.tile([F, 1], f32)
            nc.sync.dma_start(out=t_gidx[:], in_=d_gidx[:])
            nc.sync.dma_start(out=t_rl[:], in_=d_rl[:])
            nc.sync.dma_start(out=t_vals[:], in_=d_vals[:])
            nc.sync.dma_start(out=t_iota[:], in_=d_iota[:])
            nc.sync.dma_start(out=t_W[:], in_=d_W[:])
            nc.sync.dma_start(out=t_b[:], in_=d_b[:])

            for blk in range(BPC):
                t_acc = accp.tile([F, W_R], f32, space="PSUM")
                for ci in range(CPB):
                    c = blk * CPB + ci
                    t_g = gp.tile([P, F], f32)
                    nc.gpsimd.indirect_dma_start(
                        out=t_g[:],
                        out_offset=None,
                        in_=d_x[:],
                        in_offset=bass.IndirectOffsetOnAxis(
                            ap=t_gidx[:, c : c + 1], axis=0
                        ),
                    )
                    t_oh = ohp.tile([P, W_R + 1], f32)
                    nc.vector.tensor_scalar(
                        out=t_oh[:],
                        in0=t_iota[:],
                        scalar1=t_rl[:, c : c + 1],
                        scalar2=t_vals[:, c : c + 1],
                        op0=mybir.AluOpType.is_equal,
                        op1=mybir.AluOpType.mult,
                    )
                    nc.tensor.matmul(
                        out=t_acc[:],
                        lhsT=t_g[:],
                        rhs=t_oh[:, :W_R],
                        start=(ci == 0),
                        stop=(ci == CPB - 1),
                    )
                t_accs = evp.tile([F, W_R], f32)
                nc.scalar.copy(t_accs[:], t_acc[:])
                t_out = outpp.tile([F, W_R], f32, space="PSUM")
                nc.tensor.matmul(
                    out=t_out[:], lhsT=t_W[:], rhs=t_accs[:], start=True, stop=True
                )
                t_outs = evp.tile([F, W_R], f32)
                nc.scalar.add(t_outs[:], t_out[:], t_b[:, :1])
                nc.sync.dma_start(out=d_out[blk], in_=t_outs[:])

    nc.finalize()
    return nc


def _pack(rows):
    """LPT bin-packing of nodes into NBLK blocks (<=W_R nodes, <=EPB edges).

    Returns node_block[n], node_local[n]."""
    deg = np.bincount(rows, minlength=N_NODES)
    order = np.argsort(-deg, kind="stable")
    node_block = np.empty(N_NODES, dtype=np.int64)
    node_local = np.empty(N_NODES, dtype=np.int64)
    heap = [(0, b) for b in range(NBLK)]
    heapq.heapify(heap)
    bin_nodes = np.zeros(NBLK, dtype=np.int64)
    bin_edges = np.zeros(NBLK, dtype=np.int64)
    spill = []
    for n in order:
        d = int(deg[n])
        placed = False
        tmp = []
        while heap:
            e, b = heapq.heappop(heap)
            if e != bin_edges[b] or bin_nodes[b] >= W_R:
                continue  # stale or node-full entry
            if e + d <= EPB:
                node_block[n] = b
                node_local[n] = bin_nodes[b]
                bin_nodes[b] += 1
                bin_edges[b] += d
                if bin_nodes[b] < W_R:
                    heapq.heappush(heap, (int(bin_edges[b]), b))
                placed = True
                break
            else:
                tmp.append((e, b))
        for item in tmp:
            heapq.heappush(heap, item)
        if not placed:
            spill.append(n)
    if spill:
        # first-fit for spilled nodes (rare)
        for n in spill:
            d = int(deg[n])
            cand = np.where((bin_nodes < W_R) & (bin_edges + d <= EPB))[0]
            if len(cand) == 0:
                raise RuntimeError("packing failed")
            b = int(cand[0])
            node_block[n] = b
            node_local[n] = bin_nodes[b]
            bin_nodes[b] += 1
            bin_edges[b] += d
    return node_block, node_local


def kernel(x, adj_vals, adj_row, adj_col, W, b):
    rows = np.asarray(adj_row).astype(np.int64)
    cols = np.asarray(adj_col).astype(np.int64)
    vals = np.asarray(adj_vals).astype(np.float32)
    x = np.ascontiguousarray(np.asarray(x, dtype=np.float32))
    W = np.asarray(W, dtype=np.float32)
    b = np.asarray(b, dtype=np.float32)

    node_block, node_local = _pack(rows)

    # edge -> (block, slot-within-block)
    eb = node_block[rows]
    order = np.argsort(eb, kind="stable")
    eb_sorted = eb[order]
    counts = np.bincount(eb_sorted, minlength=NBLK)
    starts = np.concatenate([[0], np.cumsum(counts)[:-1]])
    pos = np.arange(N_EDGES) - np.repeat(starts, counts)

    core = eb_sorted // BPC
    chunk = (eb_sorted % BPC) * CPB + pos // P
    part = pos % P

    gidx_all = np.zeros((NCORE, P, NCH), dtype=np.int32)
    rl_all = np.zeros((NCORE, P, NCH), dtype=np.float32)
    vals_all = np.zeros((NCORE, P, NCH), dtype=np.float32)
    gidx_all[core, part, chunk] = cols[order].astype(np.int32)
    rl_all[core, part, chunk] = node_local[rows[order]].astype(np.float32)
    vals_all[core, part, chunk] = vals[order]

    iota_np = np.tile(np.arange(W_R + 1, dtype=np.float32), (P, 1)).copy()
    b2 = np.ascontiguousarray(b.reshape(F, 1))

    key = "prog"
    if key not in _cache:
        _cache[key] = _build_program()
    nc = _cache[key]

    from concourse.bass_utils import run_bass_kernel_spmd

    in_maps = []
    for k in range(NCORE):
        in_maps.append(
            {
                "x": x,
                "gidx": np.ascontiguousarray(gidx_all[k]),
                "rl": np.ascontiguousarray(rl_all[k]),
                "vals": np.ascontiguousarray(vals_all[k]),
                "iota": iota_np,
                "W": W,
                "b": b2,
            }
        )
    LAST["nc"] = nc
    LAST["in_maps"] = in_maps
    res = run_bass_kernel_spmd(nc, in_maps, list(range(NCORE)))
    LAST["res"] = res

    out_full = np.zeros((N_NODES, F), dtype=np.float32)
    nodes = np.arange(N_NODES)
    nb = node_block[nodes]
    for k in range(NCORE):
        sel = (nb // BPC) == k
        blk = (nb[sel] % BPC).astype(np.int64)
        r = node_local[nodes[sel]].astype(np.int64)
        big = res.results[k]["out"]  # [BPC, F, W_R]
        out_full[nodes[sel]] = big[blk, :, r]
    return out_full



# revision 10
# speedup vs baseline: 2448.1933x; 2448.1933x over previous
"""GNN message-passing (SpMM + dense transform) Trainium2 kernel.

out[i] = (sum_{e: row[e]==i} vals[e] * x[col[e]]) @ W + b

Strategy (8 NeuronCores, SPMD single program):
- Host packs nodes into 1600 blocks (<=64 nodes, <=640 edges each) via LPT
  bin-packing; 200 blocks per core; each block = 5 chunks of 128 edge slots.
  Within each block, edges are sorted by source (col) for HBM locality.
- x is converted to bf16 on host (halves random-gather HBM traffic; rel err
  ~0.3% vs the 2e-2 gate).
- Device processes groups of 8 blocks (40 chunks = 5120 edge slots):
  - ONE batched indirect DMA gathers 40 rows/partition ([128, 40*64] bf16).
  - TWO DVE tensor_tensor ops build all 40 one-hot chunks at once via
    stride-0 broadcast APs: eq = (iota == rl), oh = eq * vals  (bf16).
  - 40 bf16 matmuls accumulate accT[64 feats, 8*64 rows] in one PSUM bank
    (each block owns a 64-col window; start/stop per window).
  - ACT evacuates accT to bf16, one matmul with W (outT = W.T @ accT),
    ACT adds bias, one DMA out per group.
- Host unpermutes rows at the end.
"""
import sys
import heapq

for _p in ("/opt/trn_rl_repo", "/root/.axon_site/_ro/trn_rl_repo"):
    if _p not in sys.path:
        sys.path.append(_p)

import numpy as np

try:
    import ml_dtypes

    _BF16 = np.dtype(ml_dtypes.bfloat16)
except Exception:  # pragma: no cover
    _BF16 = None

N_NODES = 100000
N_EDGES = 1000000
F = 64
P = 128
W_R = 64          # rows per block
CPB = 5           # chunks per block
EPB = CPB * P     # edge slots per block = 640
NBLK = 1600       # total blocks
NCORE = 8
BPC = NBLK // NCORE   # blocks per core = 200
NCH = BPC * CPB       # chunks per core = 1000
G = 8                 # blocks per output group
NG = BPC // G         # groups per core = 25
CPG = G * CPB         # chunks per group = 40

_cache = {}
LAST = {}  # debug/profiling handle: {"nc": ..., "in_maps": [...]}


def _to_bf16(a):
    a = np.asarray(a, dtype=np.float32)
    if _BF16 is not None:
        return np.ascontiguousarray(a.astype(_BF16))
    # manual round-to-nearest-even fp32 -> bf16, kept as uint16 view
    u = a.view(np.uint32)
    rounded = ((u + 0x7FFF + ((u >> 16) & 1)) >> 16).astype(np.uint16)
    return np.ascontiguousarray(rounded)


def build_program(reps=None, timing=False):
    """Build the SPMD program. reps=None: fully unrolled single pass.
    reps=k: wraps the group loop in a hardware For_i(0, k) loop (for
    differential timing; output identical after any number of reps).
    timing=True: declare x as an internal DRAM tensor (uninitialized) so the
    timing dispatches skip the 12.8MB/core H2D; the instruction stream and
    gather addresses are identical, only the gathered values are garbage."""
    import concourse.bass as bass
    import concourse.bacc as bacc
    import concourse.mybir as mybir
    import concourse.tile as tile

    nc = bacc.Bacc(trn_type="TRN2", dynamic_dma_scratch_size=65536)
    f32 = mybir.dt.float32
    bf16 = mybir.dt.bfloat16
    if timing:
        d_x = nc.dram_tensor("xint", [N_NODES, F], bf16)
    else:
        d_x = nc.declare_dram_parameter("x", [N_NODES, F], bf16, isOutput=False)
    d_gidx = nc.declare_dram_parameter("gidx", [P, NCH], mybir.dt.int32, isOutput=False)
    d_rl = nc.declare_dram_parameter("rl", [P, NCH], bf16, isOutput=False)
    d_vals = nc.declare_dram_parameter("vals", [P, NCH], bf16, isOutput=False)
    d_iota = nc.declare_dram_parameter("iota", [P, W_R], bf16, isOutput=False)
    d_W = nc.declare_dram_parameter("W", [F, F], bf16, isOutput=False)
    d_b = nc.declare_dram_parameter("b", [F, 1], f32, isOutput=False)
    d_out = nc.declare_dram_parameter("out", [NG, F, G * W_R], f32, isOutput=True)

    with tile.TileContext(nc) as tc:
        with (
            tc.tile_pool(name="const", bufs=1) as constp,
            tc.tile_pool(name="g", bufs=3) as gp,
            tc.tile_pool(name="eq", bufs=2) as eqp,
            tc.tile_pool(name="oh", bufs=2) as ohp,
            tc.tile_pool(name="ev", bufs=3) as evp,
            tc.tile_pool(name="accp", bufs=2, space="PSUM") as accp,
            tc.tile_pool(name="outp", bufs=2, space="PSUM") as outpp,
        ):
            t_gidx = constp.tile([P, NCH], mybir.dt.int32)
            t_rl = constp.tile([P, NCH], bf16)
            t_vals = constp.tile([P, NCH], bf16)
            t_iota = constp.tile([P, W_R], bf16)
            t_W = constp.tile([F, F], bf16)
            t_b = constp.tile([F, 1], f32)
            nc.sync.dma_start(out=t_gidx[:], in_=d_gidx[:])
            nc.sync.dma_start(out=t_rl[:], in_=d_rl[:])
            nc.sync.dma_start(out=t_vals[:], in_=d_vals[:])
            nc.sync.dma_start(out=t_iota[:], in_=d_iota[:])
            nc.sync.dma_start(out=t_W[:], in_=d_W[:])
            nc.sync.dma_start(out=t_b[:], in_=d_b[:])

            def body(_i=None):
                for g in range(NG):
                    c0 = g * CPG
                    # NB: one indirect DMA per chunk — HW consumes exactly one
                    # offset per dest partition (CoreSim's multi-offset batched
                    # gather semantics do NOT match hardware). All CPG chunk
                    # gathers write disjoint slices of one group tile: fewer
                    # Tile semaphores on the Pool stream than per-chunk tiles
                    # (measured 1075us vs 1426us per iteration).
                    t_g = gp.tile([P, CPG * F], bf16, tag="gath")
                    for ci in range(CPG):
                        nc.gpsimd.indirect_dma_start(
                            out=t_g[:, ci * F : (ci + 1) * F],
                            out_offset=None,
                            in_=d_x[:],
                            in_offset=bass.IndirectOffsetOnAxis(
                                ap=t_gidx[:, c0 + ci : c0 + ci + 1], axis=0
                            ),
                        )
                    t_eq = eqp.tile([P, CPG * W_R], bf16, tag="eq")
                    t_oh = ohp.tile([P, CPG * W_R], bf16, tag="oh")
                    iota_b = t_iota[:].unsqueeze(1).broadcast_to([P, CPG, W_R])
                    rl_b = (
                        t_rl[:, c0 : c0 + CPG].unsqueeze(2).broadcast_to([P, CPG, W_R])
                    )
                    vals_b = (
                        t_vals[:, c0 : c0 + CPG]
                        .unsqueeze(2)
                        .broadcast_to([P, CPG, W_R])
                    )
                    nc.vector.tensor_tensor(
                        out=t_eq[:], in0=iota_b, in1=rl_b, op=mybir.AluOpType.is_equal
                    )
                    nc.vector.tensor_tensor(
                        out=t_oh[:], in0=t_eq[:], in1=vals_b, op=mybir.AluOpType.mult
                    )
                    t_acc = accp.tile([F, G * W_R], f32, space="PSUM", tag="acc")
                    for blk in range(G):
                        for ci in range(CPB):
                            c = blk * CPB + ci
                            nc.tensor.matmul(
                                out=t_acc[:, blk * W_R : (blk + 1) * W_R],
                                lhsT=t_g[:, c * F : (c + 1) * F],
                                rhs=t_oh[:, c * W_R : (c + 1) * W_R],
                                start=(ci == 0),
                                stop=(ci == CPB - 1),
                            )
                    t_accs = evp.tile([F, G * W_R], bf16, tag="accs")
                    nc.scalar.copy(t_accs[:], t_acc[:])
                    t_out = outpp.tile([F, G * W_R], f32, space="PSUM", tag="out")
                    nc.tensor.matmul(
                        out=t_out[:], lhsT=t_W[:], rhs=t_accs[:], start=True, stop=True
                    )
                    t_outs = evp.tile([F, G * W_R], f32, tag="outs")
                    nc.scalar.add(t_outs[:], t_out[:], t_b[:, :1])
                    nc.sync.dma_start(out=d_out[g], in_=t_outs[:])

            if reps is None:
                body()
            else:
                with tc.For_i(0, reps) as _i:
                    body(_i)

    nc.finalize()
    return nc


def _pack(rows):
    """LPT bin-packing of nodes into NBLK blocks (<=W_R nodes, <=EPB edges).

    Returns node_block[n], node_local[n]."""
    deg = np.bincount(rows, minlength=N_NODES)
    order = np.argsort(-deg, kind="stable")
    node_block = np.empty(N_NODES, dtype=np.int64)
    node_local = np.empty(N_NODES, dtype=np.int64)
    heap = [(0, b) for b in range(NBLK)]
    heapq.heapify(heap)
    bin_nodes = np.zeros(NBLK, dtype=np.int64)
    bin_edges = np.zeros(NBLK, dtype=np.int64)
    spill = []
    for n in order:
        d = int(deg[n])
        placed = False
        tmp = []
        while heap:
            e, b = heapq.heappop(heap)
            if e != bin_edges[b] or bin_nodes[b] >= W_R:
                continue  # stale or node-full entry
            if e + d <= EPB:
                node_block[n] = b
                node_local[n] = bin_nodes[b]
                bin_nodes[b] += 1
                bin_edges[b] += d
                if bin_nodes[b] < W_R:
                    heapq.heappush(heap, (int(bin_edges[b]), b))
                placed = True
                break
            else:
                tmp.append((e, b))
        for item in tmp:
            heapq.heappush(heap, item)
        if not placed:
            spill.append(n)
    if spill:
        # first-fit for spilled nodes (rare)
        for n in spill:
            d = int(deg[n])
            cand = np.where((bin_nodes < W_R) & (bin_edges + d <= EPB))[0]
            if len(cand) == 0:
                raise RuntimeError("packing failed")
            b = int(cand[0])
            node_block[n] = b
            node_local[n] = bin_nodes[b]
            bin_nodes[b] += 1
            bin_edges[b] += d
    return node_block, node_local


def prepare(x, adj_vals, adj_row, adj_col, W, b):
    """Host-side packing. Returns (in_maps, unpack_state)."""
    rows = np.asarray(adj_row).astype(np.int64)
    cols = np.asarray(adj_col).astype(np.int64)
    vals = np.asarray(adj_vals).astype(np.float32)
    x = np.ascontiguousarray(np.asarray(x, dtype=np.float32))
    W = np.asarray(W, dtype=np.float32)
    b = np.asarray(b, dtype=np.float32)

    node_block, node_local = _pack(rows)

    # edge -> (block, slot-within-block); within each block sort edges by
    # source node for DMA locality
    eb = node_block[rows]
    order = np.lexsort((cols, eb))
    eb_sorted = eb[order]
    counts = np.bincount(eb_sorted, minlength=NBLK)
    starts = np.concatenate([[0], np.cumsum(counts)[:-1]])
    pos = np.arange(N_EDGES) - np.repeat(starts, counts)

    core = eb_sorted // BPC
    chunk = (eb_sorted % BPC) * CPB + pos // P
    part = pos % P

    gidx_all = np.zeros((NCORE, P, NCH), dtype=np.int32)
    rl_all = np.zeros((NCORE, P, NCH), dtype=np.float32)
    vals_all = np.zeros((NCORE, P, NCH), dtype=np.float32)
    gidx_all[core, part, chunk] = cols[order].astype(np.int32)
    rl_all[core, part, chunk] = node_local[rows[order]].astype(np.float32)
    vals_all[core, part, chunk] = vals[order]

    x_bf = _to_bf16(x)
    iota_np = _to_bf16(np.tile(np.arange(W_R, dtype=np.float32), (P, 1)))
    W_bf = _to_bf16(W)
    b2 = np.ascontiguousarray(b.reshape(F, 1))

    in_maps = []
    for k in range(NCORE):
        in_maps.append(
            {
                "x": x_bf,
                "gidx": np.ascontiguousarray(gidx_all[k]),
                "rl": _to_bf16(rl_all[k]),
                "vals": _to_bf16(vals_all[k]),
                "iota": iota_np,
                "W": W_bf,
                "b": b2,
            }
        )
    return in_maps, (node_block, node_local)


def unpack(results, unpack_state):
    """results: list per core of {"out": [NG, F, G*W_R]} -> full [N, F]."""
    node_block, node_local = unpack_state
    out_full = np.zeros((N_NODES, F), dtype=np.float32)
    nodes = np.arange(N_NODES)
    nb = node_block[nodes]
    for k in range(NCORE):
        sel = (nb // BPC) == k
        blkc = (nb[sel] % BPC).astype(np.int64)
        r = node_local[nodes[sel]].astype(np.int64)
        big = results[k]["out"]  # [NG, F, G*W_R]
        out_full[nodes[sel]] = big[blkc // G, :, (blkc % G) * W_R + r]
    return out_full


def kernel(x, adj_vals, adj_row, adj_col, W, b):
    in_maps, unpack_state = prepare(x, adj_vals, adj_row, adj_col, W, b)

    key = "prog"
    if key not in _cache:
        _cache[key] = build_program()
    nc = _cache[key]

    from concourse.bass_utils import run_bass_kernel_spmd

    LAST["nc"] = nc
    LAST["in_maps"] = in_maps
    res = run_bass_kernel_spmd(nc, in_maps, list(range(NCORE)))
    LAST["res"] = res

    return unpack(res.results, unpack_state)
